# revision 12
# baseline (speedup 1.0000x reference)
"""Trainium2 Bass kernel for AttnProcessor self-attention (B=2,S=2048,C=1024,H=16).

Sharding: 8 cores, core c owns heads (2c, 2c+1) for both batches (tensor
parallel on the head dim for QKV). All matmul inputs are bf16 (fp32 psum
accumulation); rel err ~5e-4 vs the 2e-2 gate.

Token->core output mapping is interleaved so each 512-token q-slice (qs)
contains one 64-token block for every destination core: core c owns tokens
512*qs + 64*c .. +64 of every (b, qs). Each (b, qs) ships its attention
output in its own small AllToAll ([8,132,64] bf16 = 135KB) right after that
q-slice's PV completes - 8 pipelined collectives instead of 2 big late
ones. Payload rows 0-127 are the two heads' attnout.T in bf16; rows
128-131 carry the f32 softmax denominators (bitcast); reciprocals are
computed receiver-side with reciprocal_approx_fast. Output projection runs
per qs-pair as PE fill work inside later attention slices.

Per-core pipeline: qT/kT projections in [c'=128, s] layout (256-wide fill
units), v' in [s, 2x(64+ones)] = 130-wide layout, per-kc QK -> exp
(ScalarE, scale=1/8, no max subtraction) -> PV (accumulates attnout.T and
softmax denominators via the ones column). ScalarE runs ONLY exps; DMAs
are issued from sync (loads/recv reads/out writes) and gpsimd
(drains/collectives); DVE does casts/recips/biases/normalization.
"""
import numpy as np

import concourse.bacc as bacc
import concourse.bass as bass
import concourse.tile as tile
import concourse.tile_rust as tile_rust
from concourse import mybir
from concourse.bass_utils import run_bass_kernel_spmd

F32 = mybir.dt.float32
BF16 = mybir.dt.bfloat16
FP8E4 = mybir.dt.float8e4
FP8E5 = mybir.dt.float8e5

B, S, C, H, D = 2, 2048, 1024, 16, 64
N_CORES = 8
BS = B * S  # 4096
SCALE = 1.0 / np.sqrt(D)

# a2a chunk geometry (per destination core): 132 rows x 64 tokens bf16
CH_BF = 132 * 64          # bf16 elems per dest chunk (8448)
CH_F32 = CH_BF // 2       # f32 elems per dest chunk (4224)

_CACHE = {}


def _build():
    nc = bacc.Bacc(num_devices=N_CORES)
    hsT = nc.declare_dram_parameter("hsT", [C, BS], BF16, isOutput=False)
    wq = nc.declare_dram_parameter("wq", [C, 128], BF16, isOutput=False)
    wk = nc.declare_dram_parameter("wk", [C, 128], BF16, isOutput=False)
    wv = nc.declare_dram_parameter("wv", [C, 130], BF16, isOutput=False)
    wo = nc.declare_dram_parameter("wo", [C, C], BF16, isOutput=False)
    bqk = nc.declare_dram_parameter("bqk", [128, 2], F32, isOutput=False)
    bvb = nc.declare_dram_parameter("bvb", [1, 130], F32, isOutput=False)
    res = nc.declare_dram_parameter("res", [512, C], F32, isOutput=False)
    out1 = nc.declare_dram_parameter("out1", [256, C], F32, isOutput=True)
    out2 = nc.declare_dram_parameter("out2", [256, C], F32, isOutput=True)

    with tile.TileContext(nc) as tc:
        with (
            tc.tile_pool(name="wpool", bufs=1) as wpool,
            tc.tile_pool(name="hpool", bufs=1) as hpool,
            tc.tile_pool(name="qkpool", bufs=2) as qkpool,
            tc.tile_pool(name="ppool", bufs=4) as ppool,
            tc.tile_pool(name="spool", bufs=4) as spool,
            tc.tile_pool(name="opool", bufs=2) as opool,
            tc.tile_pool(name="psum", bufs=1, space="PSUM") as psum,
            tc.tile_pool(name="dram", bufs=1, space="DRAM") as dram,
        ):
            # ---- weight / constant / input loads (sync queue) ----
            def load_w(name, src, ncols):
                t = wpool.tile([128, 8 * ncols], BF16, tag=name)
                sap = src[:]
                nc.scalar.dma_start(
                    out=t[:],
                    in_=bass.AP(tensor=sap.tensor, offset=sap.offset,
                                ap=[[ncols, 128], [128 * ncols, 8],
                                    [1, ncols]]))
                return [t[:, ncols * cc:ncols * (cc + 1)] for cc in range(8)]

            wq_sb = load_w("wq", wq, 128)
            wk_sb = load_w("wk", wk, 128)
            wv_sb = load_w("wv", wv, 130)
            bqk_sb = wpool.tile([128, 2], F32, tag="bqk")
            nc.scalar.dma_start(out=bqk_sb[:], in_=bqk[:])
            bvb_sb = wpool.tile([128, 130], F32, tag="bvb")
            bvb_ap = bvb[:]
            nc.scalar.dma_start(
                out=bvb_sb[:],
                in_=bass.AP(tensor=bvb_ap.tensor, offset=bvb_ap.offset,
                            ap=[[0, 128], [1, 130]]))

            def emit_hsT_load(b, after=()):
                # split across sync+scalar queues so chunks land sooner
                tiles, dmas = [], []
                for cc in range(8):
                    t = hpool.tile([128, 2048], BF16, tag=f"hs{b}_{cc}",
                                   name=f"hs{b}_{cc}")
                    eng = nc.sync if cc % 2 == 0 else nc.scalar
                    d = eng.dma_start(
                        out=t[:],
                        in_=hsT[128 * cc:128 * (cc + 1),
                                2048 * b:2048 * (b + 1)])
                    for a in after:
                        tile_rust.add_dep_helper(
                            d.ins, a.ins, True, "hs1 after hs0 (bandwidth)")
                    tiles.append(t)
                    dmas.append(d)
                return tiles, dmas

            hs0, hs0_dmas = emit_hsT_load(0)
            hs1, _ = emit_hsT_load(1, after=hs0_dmas[-2:])

            a2a_in = [[dram.tile([8, 132, 64], BF16, name=f"a2ain{b}_{qs}")
                       for qs in range(4)] for b in range(2)]
            a2a_out = [[dram.tile([8, 132, 64], BF16, name=f"a2aout{b}_{qs}")
                        for qs in range(4)] for b in range(2)]

            qT, kT, vS = {}, {}, {}

            def emit_proj_qk(b, hs_sb, t_idx, j2):
                """One unit: tensor t_idx (0=q,1=k), one 256-wide s-slice."""
                if t_idx == 0:
                    if b not in qT:
                        qT[b] = qkpool.tile([128, 2048], BF16, tag="qT",
                                            name=f"qT{b}")
                    dst, w_sb = qT[b], wq_sb
                else:
                    if b not in kT:
                        kT[b] = qkpool.tile([128, 2048], BF16, tag="kT",
                                            name=f"kT{b}")
                    dst, w_sb = kT[b], wk_sb
                ps = psum.tile([128, 512], F32, tag="aux", bufs=2,
                               name=f"pqk{b}_{t_idx}_{j2}")
                sl = ps[:, 0:256]
                for cc in range(8):
                    nc.tensor.matmul(
                        sl, w_sb[cc],
                        hs_sb[cc][:, 256 * j2:256 * (j2 + 1)],
                        start=(cc == 0), stop=(cc == 7))
                nc.vector.tensor_scalar_add(
                    out=dst[:, 256 * j2:256 * (j2 + 1)], in0=sl,
                    scalar1=bqk_sb[:, t_idx:t_idx + 1])

            def emit_proj_v(b, hs_sb, i):
                """One unit: one 128-row v' s-tile i -> fp8e4 vS.
                vS layout (DoubleRow pairs): pair kc'=i//2 block at 320*kc',
                head h at +160*h, parity i%2 at +80."""
                if b not in vS:
                    vS[b] = qkpool.tile([128, 2560], FP8E4, tag="vS",
                                        name=f"vS{b}")
                ps = psum.tile([128, 512], F32, tag="aux", bufs=2,
                               name=f"pv{b}_{i}")
                sl = ps[:, 0:130]
                for cc in range(8):
                    nc.tensor.matmul(
                        sl, hs_sb[cc][:, 128 * i:128 * (i + 1)], wv_sb[cc],
                        start=(cc == 0), stop=(cc == 7))
                vt = vS[b][:]
                bvt = bvb_sb[:]
                nc.vector.tensor_tensor(
                    out=bass.AP(tensor=vt.tensor, offset=vt.offset
                                + 320 * (i // 2) + 80 * (i % 2),
                                ap=[list(vt.ap[0]), [160, 2], [1, 65]]),
                    in0=bass.AP(tensor=sl.tensor, offset=sl.offset,
                                ap=[list(sl.ap[0]), [65, 2], [1, 65]]),
                    in1=bass.AP(tensor=bvt.tensor, offset=bvt.offset,
                                ap=[list(bvt.ap[0]), [65, 2], [1, 65]]),
                    op=mybir.AluOpType.add)

            def emit_attention_qs(b, qs, fill_work):
                """One q-slice (512 q) for both heads; 16 kc steps.
                Per step: 2 fills, exp(kc), QK(kc+1), PV(kc)x2."""
                accA = psum.tile([65, 512], F32, tag="accA", bufs=1,
                                 name=f"accA_{b}_{qs}")
                accB = psum.tile([65, 512], F32, tag="accB", bufs=1,
                                 name=f"accB_{b}_{qs}")
                sc_t = {}

                def emit_qk(kc):
                    sc = psum.tile([128, 1024], F32, tag="sc", bufs=2,
                                   name=f"sc_{b}_{qs}_{kc}")
                    sc_t[kc] = sc
                    nc.tensor.matmul(
                        sc[:, 0:512],
                        kT[b][0:64, 128 * kc:128 * (kc + 1)],
                        qT[b][0:64, 512 * qs:512 * (qs + 1)],
                        start=True, stop=True)
                    nc.tensor.matmul(
                        sc[:, 512:1024],
                        kT[b][64:128, 128 * kc:128 * (kc + 1)],
                        qT[b][64:128, 512 * qs:512 * (qs + 1)],
                        start=True, stop=True)

                emit_qk(0)
                pr2 = None
                for kc in range(16):
                    for _ in range(2):
                        if fill_work:
                            fill_work.pop(0)()
                    if kc % 2 == 0:
                        pr2 = ppool.tile([128, 2048], FP8E5, tag="pr",
                                         bufs=2, name=f"pr_{b}_{qs}_{kc}")
                    nc.scalar.activation(
                        pr2[:, 1024 * (kc % 2):1024 * (kc % 2 + 1)],
                        sc_t.pop(kc)[:],
                        mybir.ActivationFunctionType.Exp,
                        scale=float(SCALE))
                    if kc < 15:
                        emit_qk(kc + 1)
                    if kc % 2 == 1:
                        kp = kc // 2
                        vt = vS[b][:]
                        prt = pr2[:]
                        for h, acc in ((0, accA), (1, accB)):
                            nc.tensor.matmul(
                                acc[:],
                                bass.AP(tensor=vt.tensor, offset=vt.offset
                                        + 320 * kp + 160 * h,
                                        ap=[list(vt.ap[0]), [80, 2],
                                            [1, 65]]),
                                bass.AP(tensor=prt.tensor, offset=prt.offset
                                        + 512 * h,
                                        ap=[list(prt.ap[0]), [1024, 2],
                                            [1, 512]]),
                                start=(kp == 0), stop=(kp == 7),
                                perf_mode=mybir.MatmulPerfMode.DoubleRow)

                # drain: cast attnout to bf16, ship per-dest + f32 sums rows
                a2a_t = a2a_in[b][qs][:]
                a2a_f = a2a_t.bitcast(F32)
                for h, acc in ((0, accA), (1, accB)):
                    st = spool.tile([64, 512], BF16, tag="st",
                                    name=f"st_{b}_{qs}_{h}")
                    nc.vector.tensor_copy(st[:], acc[0:64, :])
                    stp = st[:]
                    # payload: st[0:64, 64*d+t] -> a2a[d, 64h+r, t]
                    nc.gpsimd.dma_start(
                        out=bass.AP(tensor=a2a_t.tensor, offset=a2a_t.offset
                                    + 64 * h * 64,
                                    ap=[[64, 64], [CH_BF, 8], [1, 64]]),
                        in_=bass.AP(tensor=stp.tensor, offset=stp.offset,
                                    ap=[list(stp.ap[0]), [64, 8], [1, 64]]))
                    # f32 sums: acc[64, 64*d+t] -> f32 rows 128+2h..129+2h
                    sm_sb = spool.tile([1, 512], F32, tag="sm",
                                       name=f"sm_{b}_{qs}_{h}")
                    nc.vector.tensor_copy(sm_sb[:], acc[64:65, :])
                    sm = sm_sb[:]
                    nc.gpsimd.dma_start(
                        out=bass.AP(tensor=a2a_f.tensor, offset=a2a_f.offset
                                    + (128 + 2 * h) * 32,
                                    ap=[[CH_F32, 8], [1, 64]]),
                        in_=bass.AP(tensor=sm.tensor, offset=sm.offset,
                                    ap=[list(sm.ap[0]), [64, 8], [1, 64]]))
                nc.gpsimd.collective_compute(
                    "AllToAll", mybir.AluOpType.bypass,
                    replica_groups=[list(range(8))],
                    ins=[a2a_in[b][qs][:]], outs=[a2a_out[b][qs][:]])

            # ---- output side ----
            an_all = {}

            def emit_recv(b, p, half):
                """After A2A (b, qs=2p+half): read+normalize into an_all."""
                qs = 2 * p + half
                a2a_t = a2a_out[b][qs][:]
                a2a_f = a2a_t.bitcast(F32)
                if (b, p) not in an_all:
                    an_all[(b, p)] = opool.tile([128, 1024], BF16, tag="an",
                                                name=f"an{b}_{p}")
                raw = opool.tile([128, 512], BF16, tag="raw",
                                 name=f"raw{b}_{qs}")
                rawap = raw[:]
                nc.sync.dma_start(
                    out=bass.AP(tensor=rawap.tensor, offset=rawap.offset,
                                ap=[list(rawap.ap[0]), [64, 8], [1, 64]]),
                    in_=bass.AP(tensor=a2a_t.tensor, offset=a2a_t.offset,
                                ap=[[64, 128], [CH_BF, 8], [1, 64]]))
                sbc = opool.tile([128, 512], F32, tag="sbc",
                                 name=f"sbc{b}_{qs}")
                for h in range(2):
                    sbch = sbc[64 * h:64 * (h + 1), :]
                    nc.sync.dma_start(
                        out=bass.AP(tensor=sbch.tensor, offset=sbch.offset,
                                    ap=[list(sbch.ap[0]), [64, 8], [1, 64]]),
                        in_=bass.AP(tensor=a2a_f.tensor, offset=a2a_f.offset
                                    + (128 + 2 * h) * 32,
                                    ap=[[0, 64], [CH_F32, 8], [1, 64]]))
                rbc = opool.tile([128, 512], F32, tag="rbc",
                                 name=f"rbc{b}_{qs}")
                nc.vector.reciprocal_approx_fast(rbc[:], sbc[:])
                # an[:, 128j + 64*half + t] = raw[:, 64j+t] * rbc[:, 64j+t]
                anap = an_all[(b, p)][:]
                rbcap = rbc[:]
                nc.vector.tensor_tensor(
                    out=bass.AP(tensor=anap.tensor,
                                offset=anap.offset + 64 * half,
                                ap=[list(anap.ap[0]), [128, 8], [1, 64]]),
                    in0=bass.AP(tensor=rawap.tensor, offset=rawap.offset,
                                ap=[list(rawap.ap[0]), [64, 8], [1, 64]]),
                    in1=bass.AP(tensor=rbcap.tensor, offset=rbcap.offset,
                                ap=[list(rbcap.ap[0]), [64, 8], [1, 64]]),
                    op=mybir.AluOpType.mult)

            wo_sb = []
            res_sb = []

            def emit_out_u(b, p, co):
                """Outproj half: 512 out cols for 128 tokens of pair p."""
                an = an_all[(b, p)]
                ps = psum.tile([128, 512], F32, tag="aux", bufs=2,
                               name=f"op{b}_{p}_{co}")
                for j in range(8):
                    nc.tensor.matmul(
                        ps[:], an[:, 128 * j:128 * (j + 1)],
                        wo_sb[j][:, 512 * co:512 * (co + 1)],
                        start=(j == 0), stop=(j == 7))
                ob = opool.tile([128, 512], F32, tag="ob",
                                name=f"ob{b}_{p}_{co}")
                nc.vector.tensor_tensor(
                    out=ob[:], in0=ps[:],
                    in1=res_sb[2 * b + p][:, 512 * co:512 * (co + 1)],
                    op=mybir.AluOpType.add)
                out_t = out1 if b == 0 else out2
                nc.sync.dma_start(
                    out=out_t[128 * p:128 * (p + 1),
                              512 * co:512 * (co + 1)],
                    in_=ob[:])

            # ---------------- emission ----------------
            # prefix: enough of b0 projections for attention(b0, qs0) start
            emit_proj_qk(0, hs0, 0, 0)
            emit_proj_qk(0, hs0, 0, 1)
            emit_proj_qk(0, hs0, 1, 0)
            emit_proj_v(0, hs0, 0)
            emit_proj_v(0, hs0, 1)

            def qk_u(b, hs, t, j2):
                return lambda: emit_proj_qk(b, hs, t, j2)

            def v_u(b, hs, i):
                return lambda: emit_proj_v(b, hs, i)

            def nop():
                pass

            # qs0 fills: kT j2=1..7 early (deadline step 2*j2-2), v2..v15
            # (deadline step i-1), then qT j2=2..7 (needed by qs1+).
            fill = [qk_u(0, hs0, 1, 1), v_u(0, hs0, 2),
                    qk_u(0, hs0, 1, 2), v_u(0, hs0, 3),
                    qk_u(0, hs0, 1, 3), v_u(0, hs0, 4),
                    qk_u(0, hs0, 1, 4), v_u(0, hs0, 5),
                    qk_u(0, hs0, 1, 5), v_u(0, hs0, 6),
                    qk_u(0, hs0, 1, 6), v_u(0, hs0, 7),
                    qk_u(0, hs0, 1, 7), v_u(0, hs0, 8),
                    v_u(0, hs0, 9), v_u(0, hs0, 10),
                    v_u(0, hs0, 11), v_u(0, hs0, 12),
                    v_u(0, hs0, 13), v_u(0, hs0, 14),
                    v_u(0, hs0, 15), qk_u(0, hs0, 0, 2),
                    qk_u(0, hs0, 0, 3), qk_u(0, hs0, 0, 4),
                    qk_u(0, hs0, 0, 5), qk_u(0, hs0, 0, 6),
                    qk_u(0, hs0, 0, 7)]
            emit_attention_qs(0, 0, fill)
            assert not fill

            # wo / res load (sync queue)
            for cc in range(8):
                t = hpool.tile([128, 1024], BF16, tag=f"wo{cc}",
                               name=f"wo{cc}")
                nc.sync.dma_start(out=t[:],
                                  in_=wo[128 * cc:128 * (cc + 1), :])
                wo_sb.append(t)
            for st_i in range(4):
                t = wpool.tile([128, 1024], F32, tag=f"res{st_i}",
                               name=f"res{st_i}")
                nc.sync.dma_start(out=t[:],
                                  in_=res[128 * st_i:128 * (st_i + 1), :])
                res_sb.append(t)

            # b1 projections fill b0 qs1/qs2 (hs1 lands ~40us in)
            fill = []
            for j2 in range(8):
                fill.append(qk_u(1, hs1, 1, j2))
                fill.append(v_u(1, hs1, 2 * (j2 % 4) + (0 if j2 < 4 else 1)))
            emit_attention_qs(0, 1, fill)
            fill = []
            for j2 in range(8):
                fill.append(qk_u(1, hs1, 0, j2))
                fill.append(v_u(1, hs1,
                                8 + 2 * (j2 % 4) + (0 if j2 < 4 else 1)))
            emit_attention_qs(0, 2, fill)
            emit_attention_qs(0, 3, [])

            # outputs for b0 pair0 ready ~during b1 qs0; pair1 ~qs1;
            # b1 pair0 ~qs3; b1 pair1 = tail. Pad fills so out units
            # land mid-slice (avoid blocking the PE queue on the A2A).
            emit_recv(0, 0, 0)
            emit_recv(0, 0, 1)
            emit_attention_qs(1, 0, [])
            emit_recv(0, 1, 0)
            emit_recv(0, 1, 1)
            fill = [nop] * 6 + [lambda: emit_out_u(0, 0, 0),
                                lambda: emit_out_u(0, 0, 1)]
            emit_attention_qs(1, 1, fill)
            emit_recv(1, 0, 0)
            emit_recv(1, 0, 1)
            fill = [nop] * 6 + [lambda: emit_out_u(0, 1, 0),
                                lambda: emit_out_u(0, 1, 1)]
            emit_attention_qs(1, 2, fill)
            emit_recv(1, 1, 0)
            fill = [nop] * 6 + [lambda: emit_out_u(1, 0, 0),
                                lambda: emit_out_u(1, 0, 1)]
            emit_attention_qs(1, 3, fill)
            # keep the PE warm while the last AllToAll is in flight
            warm = psum.tile([128, 512], F32, tag="aux", bufs=2,
                             name="warm")
            for wi in range(16):
                nc.tensor.matmul(warm[:], wo_sb[0][:, 0:128],
                                 wo_sb[1][:, 0:512],
                                 start=True, stop=True,
                                 skip_group_check=True)
            emit_recv(1, 1, 1)
            emit_out_u(1, 1, 0)
            emit_out_u(1, 1, 1)
    nc.finalize()
    return nc


def _prep_inputs(hidden_states, Wq, bq, Wk, bk, Wv, bv, Wo, bo):
    import ml_dtypes
    bf16 = ml_dtypes.bfloat16
    hs = np.asarray(hidden_states, np.float32)
    hsT = np.ascontiguousarray(
        hs.transpose(2, 0, 1).reshape(C, BS)).astype(bf16)
    Wo_h = np.ascontiguousarray(np.asarray(Wo, np.float32)).astype(bf16)
    bo_f = np.asarray(bo, np.float32)
    in_maps = []
    for c in range(N_CORES):
        h0 = 2 * c
        cols = slice(64 * h0, 64 * h0 + 128)
        wv_c = np.zeros((C, 130), np.float32)
        bvb_c = np.zeros((1, 130), np.float32)
        for a in range(2):
            hd = slice(64 * (h0 + a), 64 * (h0 + a + 1))
            wv_c[:, 65 * a:65 * a + 64] = np.asarray(Wv, np.float32)[:, hd]
            bvb_c[0, 65 * a:65 * a + 64] = np.asarray(bv, np.float32)[hd]
            bvb_c[0, 65 * a + 64] = 1.0
        bqk_c = np.stack([np.asarray(bq, np.float32)[cols],
                          np.asarray(bk, np.float32)[cols]], axis=1)
        # res rows: 128*(2b+p) + 64*half + j  <->  hs[b, 512*(2p+half)+64c+j]
        res_c = np.empty((512, C), np.float32)
        for b in range(2):
            for qs in range(4):
                rows = slice(64 * (4 * b + qs), 64 * (4 * b + qs) + 64)
                toks = slice(512 * qs + 64 * c, 512 * qs + 64 * c + 64)
                res_c[rows] = hs[b, toks, :] + bo_f
        in_maps.append({
            "hsT": hsT,
            "wq": np.ascontiguousarray(
                np.asarray(Wq, np.float32)[:, cols]).astype(bf16),
            "wk": np.ascontiguousarray(
                np.asarray(Wk, np.float32)[:, cols]).astype(bf16),
            "wv": wv_c.astype(bf16),
            "wo": Wo_h,
            "bqk": np.ascontiguousarray(bqk_c),
            "bvb": bvb_c,
            "res": np.ascontiguousarray(res_c),
        })
    return in_maps


def _run(inputs, trace=False, trace_kwargs=None):
    if "nc" not in _CACHE:
        _CACHE["nc"] = _build()
    nc = _CACHE["nc"]
    in_maps = _prep_inputs(**inputs)
    r = run_bass_kernel_spmd(nc, in_maps, core_ids=list(range(N_CORES)),
                             trace=trace, **(trace_kwargs or {}))
    full = np.empty((B, S, C), np.float32)
    for c in range(N_CORES):
        for b in range(2):
            o = r.results[c]["out1" if b == 0 else "out2"]
            for qs in range(4):
                full[b, 512 * qs + 64 * c:512 * qs + 64 * c + 64, :] = \
                    o[64 * qs:64 * qs + 64]
    return full, r


def kernel(**inputs):
    full, _ = _run(inputs, trace=False)
    return full


# revision 13
# speedup vs baseline: 1.0354x; 1.0354x over previous
"""Trainium2 Bass kernel for AttnProcessor self-attention (B=2,S=2048,C=1024,H=16).

Sharding: 8 cores, core c owns heads (2c, 2c+1) for both batches (tensor
parallel on the head dim for QKV). All matmul inputs are bf16 (fp32 psum
accumulation); rel err ~5e-4 vs the 2e-2 gate.

Token->core output mapping is interleaved so each 512-token q-slice (qs)
contains one 64-token block for every destination core: core c owns tokens
512*qs + 64*c .. +64 of every (b, qs). Each (b, qs) ships its attention
output in its own small AllToAll ([8,132,64] bf16 = 135KB) right after that
q-slice's PV completes - 8 pipelined collectives instead of 2 big late
ones. Payload rows 0-127 are the two heads' attnout.T in bf16; rows
128-131 carry the f32 softmax denominators (bitcast); reciprocals are
computed receiver-side with reciprocal_approx_fast. Output projection runs
per qs-pair as PE fill work inside later attention slices.

Per-core pipeline: qT/kT projections in [c'=128, s] layout (256-wide fill
units), v' in [s, 2x(64+ones)] = 130-wide layout, per-kc QK -> exp
(ScalarE, scale=1/8, no max subtraction) -> PV (accumulates attnout.T and
softmax denominators via the ones column). ScalarE runs ONLY exps; DMAs
are issued from sync (loads/recv reads/out writes) and gpsimd
(drains/collectives); DVE does casts/recips/biases/normalization.
"""
import numpy as np

import concourse.bacc as bacc
import concourse.bass as bass
import concourse.tile as tile
import concourse.tile_rust as tile_rust
from concourse import mybir
from concourse.bass_utils import run_bass_kernel_spmd

F32 = mybir.dt.float32
BF16 = mybir.dt.bfloat16
FP8E4 = mybir.dt.float8e4
FP8E5 = mybir.dt.float8e5

B, S, C, H, D = 2, 2048, 1024, 16, 64
N_CORES = 8
BS = B * S  # 4096
SCALE = 1.0 / np.sqrt(D)

# a2a chunk geometry (per destination core): 132 rows x 64 tokens bf16
CH_BF = 132 * 64          # bf16 elems per dest chunk (8448)
CH_F32 = CH_BF // 2       # f32 elems per dest chunk (4224)

_CACHE = {}


def _build():
    nc = bacc.Bacc(num_devices=N_CORES)
    hsT = nc.declare_dram_parameter("hsT", [C, BS], BF16, isOutput=False)
    wq = nc.declare_dram_parameter("wq", [C, 128], BF16, isOutput=False)
    wk = nc.declare_dram_parameter("wk", [C, 128], BF16, isOutput=False)
    wv = nc.declare_dram_parameter("wv", [C, 130], BF16, isOutput=False)
    wo = nc.declare_dram_parameter("wo", [C, C], BF16, isOutput=False)
    bqk = nc.declare_dram_parameter("bqk", [128, 2], F32, isOutput=False)
    bvb = nc.declare_dram_parameter("bvb", [1, 130], F32, isOutput=False)
    res = nc.declare_dram_parameter("res", [512, C], BF16, isOutput=False)
    out1 = nc.declare_dram_parameter("out1", [256, C], F32, isOutput=True)
    out2 = nc.declare_dram_parameter("out2", [256, C], F32, isOutput=True)

    with tile.TileContext(nc) as tc:
        with (
            tc.tile_pool(name="wpool", bufs=1) as wpool,
            tc.tile_pool(name="hpool", bufs=1) as hpool,
            tc.tile_pool(name="qkpool", bufs=2) as qkpool,
            tc.tile_pool(name="ppool", bufs=4) as ppool,
            tc.tile_pool(name="spool", bufs=4) as spool,
            tc.tile_pool(name="opool", bufs=2) as opool,
            tc.tile_pool(name="psum", bufs=1, space="PSUM") as psum,
            tc.tile_pool(name="dram", bufs=1, space="DRAM") as dram,
        ):
            # ---- weight / constant / input loads (sync queue) ----
            def load_w(name, src, ncols):
                t = wpool.tile([128, 8 * ncols], BF16, tag=name)
                sap = src[:]
                nc.gpsimd.dma_start(
                    out=t[:],
                    in_=bass.AP(tensor=sap.tensor, offset=sap.offset,
                                ap=[[ncols, 128], [128 * ncols, 8],
                                    [1, ncols]]))
                return [t[:, ncols * cc:ncols * (cc + 1)] for cc in range(8)]

            wq_sb = load_w("wq", wq, 128)
            wk_sb = load_w("wk", wk, 128)
            wv_sb = load_w("wv", wv, 130)
            bqk_sb = wpool.tile([128, 2], F32, tag="bqk")
            nc.gpsimd.dma_start(out=bqk_sb[:], in_=bqk[:])
            bvb_sb = wpool.tile([128, 130], F32, tag="bvb")
            bvb_ap = bvb[:]
            nc.gpsimd.dma_start(
                out=bvb_sb[:],
                in_=bass.AP(tensor=bvb_ap.tensor, offset=bvb_ap.offset,
                            ap=[[0, 128], [1, 130]]))

            def emit_hsT_load(b, engs, after=()):
                tiles, dmas = [], []
                for cc in range(8):
                    t = hpool.tile([128, 2048], BF16, tag=f"hs{b}_{cc}",
                                   name=f"hs{b}_{cc}")
                    d = engs[cc % len(engs)].dma_start(
                        out=t[:],
                        in_=hsT[128 * cc:128 * (cc + 1),
                                2048 * b:2048 * (b + 1)])
                    for a in after:
                        tile_rust.add_dep_helper(
                            d.ins, a.ins, True, "hs1 after hs0 (bandwidth)")
                    tiles.append(t)
                    dmas.append(d)
                return tiles, dmas

            hs0, hs0_dmas = emit_hsT_load(0, [nc.sync, nc.scalar, nc.gpsimd])
            hs1, _ = emit_hsT_load(1, [nc.sync, nc.scalar],
                                   after=hs0_dmas[-3:])

            a2a_in = [[dram.tile([8, 132, 64], BF16, name=f"a2ain{b}_{qs}")
                       for qs in range(4)] for b in range(2)]
            a2a_out = [[dram.tile([8, 132, 64], BF16, name=f"a2aout{b}_{qs}")
                        for qs in range(4)] for b in range(2)]

            qT, kT, vS = {}, {}, {}

            def emit_proj_qk(b, hs_sb, t_idx, j2):
                """One unit: tensor t_idx (0=q,1=k), one 256-wide s-slice."""
                if t_idx == 0:
                    if b not in qT:
                        qT[b] = qkpool.tile([128, 2048], BF16, tag="qT",
                                            name=f"qT{b}")
                    dst, w_sb = qT[b], wq_sb
                else:
                    if b not in kT:
                        kT[b] = qkpool.tile([128, 2048], BF16, tag="kT",
                                            name=f"kT{b}")
                    dst, w_sb = kT[b], wk_sb
                ps = psum.tile([128, 512], F32, tag="aux", bufs=2,
                               name=f"pqk{b}_{t_idx}_{j2}")
                sl = ps[:, 0:256]
                for cc in range(8):
                    nc.tensor.matmul(
                        sl, w_sb[cc],
                        hs_sb[cc][:, 256 * j2:256 * (j2 + 1)],
                        start=(cc == 0), stop=(cc == 7))
                nc.vector.tensor_scalar_add(
                    out=dst[:, 256 * j2:256 * (j2 + 1)], in0=sl,
                    scalar1=bqk_sb[:, t_idx:t_idx + 1])

            def emit_proj_v(b, hs_sb, i):
                """One unit: one 128-row v' s-tile i -> fp8e4 vS.
                vS layout (DoubleRow pairs): pair kc'=i//2 block at 320*kc',
                head h at +160*h, parity i%2 at +80."""
                if b not in vS:
                    vS[b] = qkpool.tile([128, 2560], FP8E4, tag="vS",
                                        name=f"vS{b}")
                ps = psum.tile([128, 512], F32, tag="aux", bufs=2,
                               name=f"pv{b}_{i}")
                sl = ps[:, 0:130]
                for cc in range(8):
                    nc.tensor.matmul(
                        sl, hs_sb[cc][:, 128 * i:128 * (i + 1)], wv_sb[cc],
                        start=(cc == 0), stop=(cc == 7))
                vt = vS[b][:]
                bvt = bvb_sb[:]
                nc.vector.tensor_tensor(
                    out=bass.AP(tensor=vt.tensor, offset=vt.offset
                                + 320 * (i // 2) + 80 * (i % 2),
                                ap=[list(vt.ap[0]), [160, 2], [1, 65]]),
                    in0=bass.AP(tensor=sl.tensor, offset=sl.offset,
                                ap=[list(sl.ap[0]), [65, 2], [1, 65]]),
                    in1=bass.AP(tensor=bvt.tensor, offset=bvt.offset,
                                ap=[list(bvt.ap[0]), [65, 2], [1, 65]]),
                    op=mybir.AluOpType.add)

            def emit_attention_qs(b, qs, fill_work):
                """One q-slice (512 q) for both heads; 16 kc steps.
                Per step: 2 fills, exp(kc), QK(kc+1), PV(kc)x2."""
                accA = psum.tile([65, 512], F32, tag="accA", bufs=1,
                                 name=f"accA_{b}_{qs}")
                accB = psum.tile([65, 512], F32, tag="accB", bufs=1,
                                 name=f"accB_{b}_{qs}")
                sc_t = {}

                def emit_qk(kc):
                    sc = psum.tile([128, 1024], F32, tag="sc", bufs=2,
                                   name=f"sc_{b}_{qs}_{kc}")
                    sc_t[kc] = sc
                    nc.tensor.matmul(
                        sc[:, 0:512],
                        kT[b][0:64, 128 * kc:128 * (kc + 1)],
                        qT[b][0:64, 512 * qs:512 * (qs + 1)],
                        start=True, stop=True)
                    nc.tensor.matmul(
                        sc[:, 512:1024],
                        kT[b][64:128, 128 * kc:128 * (kc + 1)],
                        qT[b][64:128, 512 * qs:512 * (qs + 1)],
                        start=True, stop=True)

                emit_qk(0)
                pr2 = None
                for kc in range(16):
                    for _ in range(2):
                        if fill_work:
                            fill_work.pop(0)()
                    if kc % 2 == 0:
                        pr2 = ppool.tile([128, 2048], FP8E5, tag="pr",
                                         bufs=2, name=f"pr_{b}_{qs}_{kc}")
                    nc.scalar.activation(
                        pr2[:, 1024 * (kc % 2):1024 * (kc % 2 + 1)],
                        sc_t.pop(kc)[:],
                        mybir.ActivationFunctionType.Exp,
                        scale=float(SCALE))
                    if kc < 15:
                        emit_qk(kc + 1)
                    if kc % 2 == 1:
                        kp = kc // 2
                        vt = vS[b][:]
                        prt = pr2[:]
                        for h, acc in ((0, accA), (1, accB)):
                            nc.tensor.matmul(
                                acc[:],
                                bass.AP(tensor=vt.tensor, offset=vt.offset
                                        + 320 * kp + 160 * h,
                                        ap=[list(vt.ap[0]), [80, 2],
                                            [1, 65]]),
                                bass.AP(tensor=prt.tensor, offset=prt.offset
                                        + 512 * h,
                                        ap=[list(prt.ap[0]), [1024, 2],
                                            [1, 512]]),
                                start=(kp == 0), stop=(kp == 7),
                                perf_mode=mybir.MatmulPerfMode.DoubleRow)

                # drain: cast attnout to bf16, ship per-dest + f32 sums rows
                a2a_t = a2a_in[b][qs][:]
                a2a_f = a2a_t.bitcast(F32)
                for h, acc in ((0, accA), (1, accB)):
                    st = spool.tile([64, 512], BF16, tag="st",
                                    name=f"st_{b}_{qs}_{h}")
                    nc.vector.tensor_copy(st[:], acc[0:64, :])
                    stp = st[:]
                    # payload: st[0:64, 64*d+t] -> a2a[d, 64h+r, t]
                    nc.gpsimd.dma_start(
                        out=bass.AP(tensor=a2a_t.tensor, offset=a2a_t.offset
                                    + 64 * h * 64,
                                    ap=[[64, 64], [CH_BF, 8], [1, 64]]),
                        in_=bass.AP(tensor=stp.tensor, offset=stp.offset,
                                    ap=[list(stp.ap[0]), [64, 8], [1, 64]]))
                    # f32 sums: acc[64, 64*d+t] -> f32 rows 128+2h..129+2h
                    sm_sb = spool.tile([1, 512], F32, tag="sm",
                                       name=f"sm_{b}_{qs}_{h}")
                    nc.vector.tensor_copy(sm_sb[:], acc[64:65, :])
                    sm = sm_sb[:]
                    nc.gpsimd.dma_start(
                        out=bass.AP(tensor=a2a_f.tensor, offset=a2a_f.offset
                                    + (128 + 2 * h) * 32,
                                    ap=[[CH_F32, 8], [1, 64]]),
                        in_=bass.AP(tensor=sm.tensor, offset=sm.offset,
                                    ap=[list(sm.ap[0]), [64, 8], [1, 64]]))
                nc.gpsimd.collective_compute(
                    "AllToAll", mybir.AluOpType.bypass,
                    replica_groups=[list(range(8))],
                    ins=[a2a_in[b][qs][:]], outs=[a2a_out[b][qs][:]])

            # ---- output side ----
            an_all = {}

            def emit_recv(b, p, half):
                """After A2A (b, qs=2p+half): read+normalize into an_all."""
                qs = 2 * p + half
                a2a_t = a2a_out[b][qs][:]
                a2a_f = a2a_t.bitcast(F32)
                if (b, p) not in an_all:
                    an_all[(b, p)] = opool.tile([128, 1024], BF16, tag="an",
                                                name=f"an{b}_{p}")
                raw = opool.tile([128, 512], BF16, tag="raw",
                                 name=f"raw{b}_{qs}")
                rawap = raw[:]
                nc.sync.dma_start(
                    out=bass.AP(tensor=rawap.tensor, offset=rawap.offset,
                                ap=[list(rawap.ap[0]), [64, 8], [1, 64]]),
                    in_=bass.AP(tensor=a2a_t.tensor, offset=a2a_t.offset,
                                ap=[[64, 128], [CH_BF, 8], [1, 64]]))
                sbc = opool.tile([128, 512], F32, tag="sbc",
                                 name=f"sbc{b}_{qs}")
                for h in range(2):
                    sbch = sbc[64 * h:64 * (h + 1), :]
                    nc.sync.dma_start(
                        out=bass.AP(tensor=sbch.tensor, offset=sbch.offset,
                                    ap=[list(sbch.ap[0]), [64, 8], [1, 64]]),
                        in_=bass.AP(tensor=a2a_f.tensor, offset=a2a_f.offset
                                    + (128 + 2 * h) * 32,
                                    ap=[[0, 64], [CH_F32, 8], [1, 64]]))
                rbc = opool.tile([128, 512], F32, tag="rbc",
                                 name=f"rbc{b}_{qs}")
                nc.vector.reciprocal_approx_fast(rbc[:], sbc[:])
                # an[:, 128j + 64*half + t] = raw[:, 64j+t] * rbc[:, 64j+t]
                anap = an_all[(b, p)][:]
                rbcap = rbc[:]
                nc.vector.tensor_tensor(
                    out=bass.AP(tensor=anap.tensor,
                                offset=anap.offset + 64 * half,
                                ap=[list(anap.ap[0]), [128, 8], [1, 64]]),
                    in0=bass.AP(tensor=rawap.tensor, offset=rawap.offset,
                                ap=[list(rawap.ap[0]), [64, 8], [1, 64]]),
                    in1=bass.AP(tensor=rbcap.tensor, offset=rbcap.offset,
                                ap=[list(rbcap.ap[0]), [64, 8], [1, 64]]),
                    op=mybir.AluOpType.mult)

            wo_sb = []
            res_sb = []

            def emit_out_u(b, p, co):
                """Outproj half: 512 out cols for 128 tokens of pair p."""
                an = an_all[(b, p)]
                ps = psum.tile([128, 512], F32, tag="aux", bufs=2,
                               name=f"op{b}_{p}_{co}")
                for j in range(8):
                    nc.tensor.matmul(
                        ps[:], an[:, 128 * j:128 * (j + 1)],
                        wo_sb[j][:, 512 * co:512 * (co + 1)],
                        start=(j == 0), stop=(j == 7))
                ob = opool.tile([128, 512], F32, tag="ob",
                                name=f"ob{b}_{p}_{co}")
                nc.vector.tensor_tensor(
                    out=ob[:], in0=ps[:],
                    in1=res_sb[2 * b + p][:, 512 * co:512 * (co + 1)],
                    op=mybir.AluOpType.add)
                out_t = out1 if b == 0 else out2
                nc.sync.dma_start(
                    out=out_t[128 * p:128 * (p + 1),
                              512 * co:512 * (co + 1)],
                    in_=ob[:])

            # ---------------- emission ----------------
            # prefix: enough of b0 projections for attention(b0, qs0) start
            emit_proj_qk(0, hs0, 0, 0)
            emit_proj_qk(0, hs0, 0, 1)
            emit_proj_qk(0, hs0, 1, 0)
            emit_proj_v(0, hs0, 0)
            emit_proj_v(0, hs0, 1)

            def qk_u(b, hs, t, j2):
                return lambda: emit_proj_qk(b, hs, t, j2)

            def v_u(b, hs, i):
                return lambda: emit_proj_v(b, hs, i)

            def nop():
                pass

            # qs0 fills: kT j2=1..7 early (deadline step 2*j2-2), v2..v15
            # (deadline step i-1), then qT j2=2..7 (needed by qs1+).
            fill = [qk_u(0, hs0, 1, 1), v_u(0, hs0, 2),
                    qk_u(0, hs0, 1, 2), v_u(0, hs0, 3),
                    qk_u(0, hs0, 1, 3), v_u(0, hs0, 4),
                    qk_u(0, hs0, 1, 4), v_u(0, hs0, 5),
                    qk_u(0, hs0, 1, 5), v_u(0, hs0, 6),
                    qk_u(0, hs0, 1, 6), v_u(0, hs0, 7),
                    qk_u(0, hs0, 1, 7), v_u(0, hs0, 8),
                    v_u(0, hs0, 9), v_u(0, hs0, 10),
                    v_u(0, hs0, 11), v_u(0, hs0, 12),
                    v_u(0, hs0, 13), v_u(0, hs0, 14),
                    v_u(0, hs0, 15), qk_u(0, hs0, 0, 2),
                    qk_u(0, hs0, 0, 3), qk_u(0, hs0, 0, 4),
                    qk_u(0, hs0, 0, 5), qk_u(0, hs0, 0, 6),
                    qk_u(0, hs0, 0, 7)]
            emit_attention_qs(0, 0, fill)
            assert not fill

            # wo / res load (sync queue)
            for cc in range(8):
                t = hpool.tile([128, 1024], BF16, tag=f"wo{cc}",
                               name=f"wo{cc}")
                nc.sync.dma_start(out=t[:],
                                  in_=wo[128 * cc:128 * (cc + 1), :])
                wo_sb.append(t)
            for st_i in range(4):
                t = wpool.tile([128, 1024], BF16, tag=f"res{st_i}",
                               name=f"res{st_i}")
                nc.sync.dma_start(out=t[:],
                                  in_=res[128 * st_i:128 * (st_i + 1), :])
                res_sb.append(t)

            # b1 projections fill b0 qs1/qs2 (hs1 lands ~40us in)
            fill = []
            for j2 in range(8):
                fill.append(qk_u(1, hs1, 1, j2))
                fill.append(v_u(1, hs1, 2 * (j2 % 4) + (0 if j2 < 4 else 1)))
            emit_attention_qs(0, 1, fill)
            fill = []
            for j2 in range(8):
                fill.append(qk_u(1, hs1, 0, j2))
                fill.append(v_u(1, hs1,
                                8 + 2 * (j2 % 4) + (0 if j2 < 4 else 1)))
            emit_attention_qs(0, 2, fill)
            emit_attention_qs(0, 3, [])

            # outputs for b0 pair0 ready ~during b1 qs0; pair1 ~qs1;
            # b1 pair0 ~qs3; b1 pair1 = tail. Pad fills so out units
            # land mid-slice (avoid blocking the PE queue on the A2A).
            emit_attention_qs(1, 0, [])
            emit_recv(0, 0, 0)
            emit_recv(0, 0, 1)
            emit_attention_qs(1, 1, [])
            emit_recv(0, 1, 0)
            emit_recv(0, 1, 1)
            fill = [nop] * 12 + [lambda: emit_out_u(0, 0, 0),
                                 lambda: emit_out_u(0, 0, 1)]
            emit_attention_qs(1, 2, fill)
            emit_recv(1, 0, 0)
            emit_recv(1, 0, 1)
            fill = [nop] * 12 + [lambda: emit_out_u(0, 1, 0),
                                 lambda: emit_out_u(0, 1, 1)]
            emit_attention_qs(1, 3, fill)
            # tail: out(1,0) doubles as PE warm-keeper during last A2A
            emit_out_u(1, 0, 0)
            emit_out_u(1, 0, 1)
            warm = psum.tile([128, 512], F32, tag="aux", bufs=2,
                             name="warm")
            for wi in range(10):
                nc.tensor.matmul(warm[:], wo_sb[0][:, 0:128],
                                 wo_sb[1][:, 0:512],
                                 start=True, stop=True,
                                 skip_group_check=True)
            emit_recv(1, 1, 0)
            emit_recv(1, 1, 1)
            emit_out_u(1, 1, 0)
            emit_out_u(1, 1, 1)
    nc.finalize()
    return nc


def _prep_inputs(hidden_states, Wq, bq, Wk, bk, Wv, bv, Wo, bo):
    import ml_dtypes
    bf16 = ml_dtypes.bfloat16
    hs = np.asarray(hidden_states, np.float32)
    hsT = np.ascontiguousarray(
        hs.transpose(2, 0, 1).reshape(C, BS)).astype(bf16)
    Wo_h = np.ascontiguousarray(np.asarray(Wo, np.float32)).astype(bf16)
    bo_f = np.asarray(bo, np.float32)
    in_maps = []
    for c in range(N_CORES):
        h0 = 2 * c
        cols = slice(64 * h0, 64 * h0 + 128)
        wv_c = np.zeros((C, 130), np.float32)
        bvb_c = np.zeros((1, 130), np.float32)
        for a in range(2):
            hd = slice(64 * (h0 + a), 64 * (h0 + a + 1))
            wv_c[:, 65 * a:65 * a + 64] = np.asarray(Wv, np.float32)[:, hd]
            bvb_c[0, 65 * a:65 * a + 64] = np.asarray(bv, np.float32)[hd]
            bvb_c[0, 65 * a + 64] = 1.0
        bqk_c = np.stack([np.asarray(bq, np.float32)[cols],
                          np.asarray(bk, np.float32)[cols]], axis=1)
        # res rows: 128*(2b+p) + 64*half + j  <->  hs[b, 512*(2p+half)+64c+j]
        res_c = np.empty((512, C), np.float32)
        for b in range(2):
            for qs in range(4):
                rows = slice(64 * (4 * b + qs), 64 * (4 * b + qs) + 64)
                toks = slice(512 * qs + 64 * c, 512 * qs + 64 * c + 64)
                res_c[rows] = hs[b, toks, :] + bo_f
        in_maps.append({
            "hsT": hsT,
            "wq": np.ascontiguousarray(
                np.asarray(Wq, np.float32)[:, cols]).astype(bf16),
            "wk": np.ascontiguousarray(
                np.asarray(Wk, np.float32)[:, cols]).astype(bf16),
            "wv": wv_c.astype(bf16),
            "wo": Wo_h,
            "bqk": np.ascontiguousarray(bqk_c),
            "bvb": bvb_c,
            "res": np.ascontiguousarray(res_c).astype(bf16),
        })
    return in_maps


def _run(inputs, trace=False, trace_kwargs=None):
    if "nc" not in _CACHE:
        _CACHE["nc"] = _build()
    nc = _CACHE["nc"]
    in_maps = _prep_inputs(**inputs)
    r = run_bass_kernel_spmd(nc, in_maps, core_ids=list(range(N_CORES)),
                             trace=trace, **(trace_kwargs or {}))
    full = np.empty((B, S, C), np.float32)
    for c in range(N_CORES):
        for b in range(2):
            o = r.results[c]["out1" if b == 0 else "out2"]
            for qs in range(4):
                full[b, 512 * qs + 64 * c:512 * qs + 64 * c + 64, :] = \
                    o[64 * qs:64 * qs + 64]
    return full, r


def kernel(**inputs):
    full, _ = _run(inputs, trace=False)
    return full


# revision 16
# speedup vs baseline: 1.0512x; 1.0152x over previous
"""Trainium2 Bass kernel for AttnProcessor self-attention (B=2,S=2048,C=1024,H=16).

Sharding: 8 cores, core c owns heads (2c, 2c+1) for both batches (tensor
parallel on the head dim for QKV). Projections run in fp8e4 DoubleRow
(hs and x32-scaled weights; the ones-column carries 32.0 so softmax
normalization absorbs the scale); QK runs bf16; PV runs fp8 DoubleRow
(probs e5m2, v e4m3); output projection bf16. rel err ~8e-3 vs 2e-2 gate.

Token->core output mapping is interleaved so each 512-token q-slice (qs)
contains one 64-token block for every destination core: core c owns tokens
512*qs + 64*c .. +64 of every (b, qs). Each (b, qs) ships its attention
output in its own small AllToAll ([8,132,64] bf16; rows 128-131 carry the
f32 softmax sums, bitcast) right after that q-slice's PV completes - 8
pipelined collectives. Reciprocals run receiver-side with
reciprocal_approx_fast; output projection runs per qs-pair as PE fill
work inside later attention slices / the tail.

b0's hs arrives in 512-token column slices so the projection pipeline
starts after ~1/4 of the load. ScalarE runs ONLY exps (plus a few early
load DMAs); DMAs are issued from sync/scalar/gpsimd rings; DVE does
casts/recips/biases/normalization.
"""
import numpy as np

import concourse.bacc as bacc
import concourse.bass as bass
import concourse.tile as tile
import concourse.tile_rust as tile_rust
from concourse import mybir
from concourse.bass_utils import run_bass_kernel_spmd

F32 = mybir.dt.float32
BF16 = mybir.dt.bfloat16
FP8E4 = mybir.dt.float8e4
FP8E5 = mybir.dt.float8e5

B, S, C, H, D = 2, 2048, 1024, 16, 64
N_CORES = 8
BS = B * S  # 4096
SCALE = 1.0 / np.sqrt(D)
WS = 32.0  # fp8 projection weight scale (weights are sigma=1/32)

# a2a chunk geometry (per destination core): 132 rows x 64 tokens bf16
CH_BF = 132 * 64          # bf16 elems per dest chunk (8448)
CH_F32 = CH_BF // 2       # f32 elems per dest chunk (4224)

_CACHE = {}
DR = mybir.MatmulPerfMode.DoubleRow


def _build():
    nc = bacc.Bacc(num_devices=N_CORES)
    hsT = nc.declare_dram_parameter("hsT", [C, BS], FP8E4, isOutput=False)
    wq = nc.declare_dram_parameter("wq", [128, 1024], FP8E4, isOutput=False)
    wk = nc.declare_dram_parameter("wk", [128, 1024], FP8E4, isOutput=False)
    wv = nc.declare_dram_parameter("wv", [128, 1152], FP8E4, isOutput=False)
    wo = nc.declare_dram_parameter("wo", [C, C], BF16, isOutput=False)
    bqk = nc.declare_dram_parameter("bqk", [128, 2], F32, isOutput=False)
    bvb = nc.declare_dram_parameter("bvb", [1, 130], F32, isOutput=False)
    res = nc.declare_dram_parameter("res", [512, C], BF16, isOutput=False)
    out1 = nc.declare_dram_parameter("out1", [256, C], F32, isOutput=True)
    out2 = nc.declare_dram_parameter("out2", [256, C], F32, isOutput=True)

    with tile.TileContext(nc) as tc:
        with (
            tc.tile_pool(name="wpool", bufs=1) as wpool,
            tc.tile_pool(name="hpool", bufs=1) as hpool,
            tc.tile_pool(name="qkpool", bufs=2) as qkpool,
            tc.tile_pool(name="ppool", bufs=4) as ppool,
            tc.tile_pool(name="spool", bufs=4) as spool,
            tc.tile_pool(name="opool", bufs=2) as opool,
            tc.tile_pool(name="psum", bufs=1, space="PSUM") as psum,
            tc.tile_pool(name="dram", bufs=1, space="DRAM") as dram,
        ):
            # ---- weight / constant loads ----
            wq_sb = wpool.tile([128, 1024], FP8E4, tag="wq")
            nc.sync.dma_start(out=wq_sb[:], in_=wq[:])
            wk_sb = wpool.tile([128, 1024], FP8E4, tag="wk")
            nc.scalar.dma_start(out=wk_sb[:], in_=wk[:])
            wv_sb = wpool.tile([128, 1152], FP8E4, tag="wv")
            nc.gpsimd.dma_start(out=wv_sb[:], in_=wv[:])
            bqk_sb = wpool.tile([128, 2], F32, tag="bqk")
            nc.gpsimd.dma_start(out=bqk_sb[:], in_=bqk[:])
            bvb_sb = wpool.tile([128, 130], F32, tag="bvb")
            bvb_ap = bvb[:]
            nc.gpsimd.dma_start(
                out=bvb_sb[:],
                in_=bass.AP(tensor=bvb_ap.tensor, offset=bvb_ap.offset,
                            ap=[[0, 128], [1, 130]]))

            # b0 hs: chunk-pair tiles [128, 2x512] per (mp, j4), streamed
            # token-slice-major across 3 rings
            rings = [nc.sync, nc.scalar, nc.gpsimd]
            hs0 = [[None] * 4 for _ in range(4)]
            hs0_dmas = []
            n = 0
            for j4 in range(4):
                for mp in range(4):
                    t = hpool.tile([128, 1024], FP8E4, tag=f"hs0_{mp}_{j4}",
                                   name=f"hs0_{mp}_{j4}")
                    for i in range(2):
                        d = rings[n % 3].dma_start(
                            out=t[:, 512 * i:512 * (i + 1)],
                            in_=hsT[128 * (2 * mp + i):
                                    128 * (2 * mp + i + 1),
                                    512 * j4:512 * (j4 + 1)])
                        hs0_dmas.append(d)
                        n += 1
                    hs0[mp][j4] = t
            hs1 = []
            for mp in range(4):
                t = hpool.tile([128, 4096], FP8E4, tag=f"hs1_{mp}",
                               name=f"hs1_{mp}")
                for i in range(2):
                    d = [nc.sync, nc.scalar][(mp + i) % 2].dma_start(
                        out=t[:, 2048 * i:2048 * (i + 1)],
                        in_=hsT[128 * (2 * mp + i):128 * (2 * mp + i + 1),
                                2048:4096])
                    for a in hs0_dmas[-3:]:
                        tile_rust.add_dep_helper(
                            d.ins, a.ins, True, "hs1 after hs0 (bandwidth)")
                hs1.append(t)

            a2a_in = [[dram.tile([8, 132, 64], BF16, name=f"a2ain{b}_{qs}")
                       for qs in range(4)] for b in range(2)]
            a2a_out = [[dram.tile([8, 132, 64], BF16, name=f"a2aout{b}_{qs}")
                        for qs in range(4)] for b in range(2)]

            qT, kT, vS = {}, {}, {}

            def hs_rhs(b, mp, col, width):
                """fp8 DR AP for hs chunk-pair mp, token cols [col,col+w)."""
                if b == 0:
                    t = hs0[mp][col // 512][:]
                    off = col % 512
                    return bass.AP(tensor=t.tensor, offset=t.offset + off,
                                   ap=[list(t.ap[0]), [512, 2], [1, width]])
                t = hs1[mp][:]
                return bass.AP(tensor=t.tensor, offset=t.offset + col,
                               ap=[list(t.ap[0]), [2048, 2], [1, width]])

            def emit_proj_qk(b, t_idx, j2):
                """One unit: tensor t_idx (0=q,1=k), one 256-wide s-slice.
                fp8 DoubleRow over chunk pairs."""
                if t_idx == 0:
                    if b not in qT:
                        qT[b] = qkpool.tile([128, 2048], BF16, tag="qT",
                                            name=f"qT{b}")
                    dst, w_sb = qT[b], wq_sb
                else:
                    if b not in kT:
                        kT[b] = qkpool.tile([128, 2048], BF16, tag="kT",
                                            name=f"kT{b}")
                    dst, w_sb = kT[b], wk_sb
                ps = psum.tile([128, 512], F32, tag="aux", bufs=2,
                               name=f"pqk{b}_{t_idx}_{j2}")
                sl = ps[:, 0:256]
                wap = w_sb[:]
                for mp in range(4):
                    nc.tensor.matmul(
                        sl,
                        bass.AP(tensor=wap.tensor,
                                offset=wap.offset + 256 * mp,
                                ap=[list(wap.ap[0]), [128, 2], [1, 128]]),
                        hs_rhs(b, mp, 256 * j2, 256),
                        start=(mp == 0), stop=(mp == 3), perf_mode=DR)
                nc.vector.tensor_scalar_add(
                    out=dst[:, 256 * j2:256 * (j2 + 1)], in0=sl,
                    scalar1=bqk_sb[:, t_idx:t_idx + 1])

            def emit_proj_v(b, i):
                """One unit: one 128-row v' s-tile i -> fp8e4 vS (x32).
                vS layout (PV DoubleRow pairs): pair kc'=i//2 block at
                320*kc', head h at +160*h, parity i%2 at +80."""
                if b not in vS:
                    vS[b] = qkpool.tile([128, 2560], FP8E4, tag="vS",
                                        name=f"vS{b}")
                ps = psum.tile([128, 512], F32, tag="aux", bufs=2,
                               name=f"pv{b}_{i}")
                sl = ps[:, 0:144]
                wap = wv_sb[:]
                for mp in range(4):
                    nc.tensor.matmul(
                        sl,
                        hs_rhs(b, mp, 128 * i, 128),
                        bass.AP(tensor=wap.tensor,
                                offset=wap.offset + 288 * mp,
                                ap=[list(wap.ap[0]), [144, 2], [1, 144]]),
                        start=(mp == 0), stop=(mp == 3), perf_mode=DR)
                vt = vS[b][:]
                bvt = bvb_sb[:]
                slb = ps[:, 0:130]
                nc.vector.tensor_tensor(
                    out=bass.AP(tensor=vt.tensor, offset=vt.offset
                                + 320 * (i // 2) + 80 * (i % 2),
                                ap=[list(vt.ap[0]), [160, 2], [1, 65]]),
                    in0=bass.AP(tensor=slb.tensor, offset=slb.offset,
                                ap=[list(slb.ap[0]), [65, 2], [1, 65]]),
                    in1=bass.AP(tensor=bvt.tensor, offset=bvt.offset,
                                ap=[list(bvt.ap[0]), [65, 2], [1, 65]]),
                    op=mybir.AluOpType.add)

            def emit_attention_qs(b, qs, fill_work):
                """One q-slice (512 q) for both heads; 16 kc steps.
                Per step: 2 fills, exp(kc), QK(kc+1); PV (DoubleRow,
                paired kc blocks) after odd kc."""
                accA = psum.tile([65, 512], F32, tag="accA", bufs=1,
                                 name=f"accA_{b}_{qs}")
                accB = psum.tile([65, 512], F32, tag="accB", bufs=1,
                                 name=f"accB_{b}_{qs}")
                sc_t = {}

                def emit_qk(kc):
                    sc = psum.tile([128, 1024], F32, tag="sc", bufs=2,
                                   name=f"sc_{b}_{qs}_{kc}")
                    sc_t[kc] = sc
                    nc.tensor.matmul(
                        sc[:, 0:512],
                        kT[b][0:64, 128 * kc:128 * (kc + 1)],
                        qT[b][0:64, 512 * qs:512 * (qs + 1)],
                        start=True, stop=True)
                    nc.tensor.matmul(
                        sc[:, 512:1024],
                        kT[b][64:128, 128 * kc:128 * (kc + 1)],
                        qT[b][64:128, 512 * qs:512 * (qs + 1)],
                        start=True, stop=True)

                emit_qk(0)
                pr2 = None
                for kc in range(16):
                    for _ in range(2):
                        if fill_work:
                            fill_work.pop(0)()
                    if kc % 2 == 0:
                        pr2 = ppool.tile([128, 2048], FP8E5, tag="pr",
                                         bufs=2, name=f"pr_{b}_{qs}_{kc}")
                    nc.scalar.activation(
                        pr2[:, 1024 * (kc % 2):1024 * (kc % 2 + 1)],
                        sc_t.pop(kc)[:],
                        mybir.ActivationFunctionType.Exp,
                        scale=float(SCALE / (WS * WS)))
                    if kc < 15:
                        emit_qk(kc + 1)
                    if kc % 2 == 1:
                        kp = kc // 2
                        vt = vS[b][:]
                        prt = pr2[:]
                        for h, acc in ((0, accA), (1, accB)):
                            nc.tensor.matmul(
                                acc[:],
                                bass.AP(tensor=vt.tensor, offset=vt.offset
                                        + 320 * kp + 160 * h,
                                        ap=[list(vt.ap[0]), [80, 2],
                                            [1, 65]]),
                                bass.AP(tensor=prt.tensor,
                                        offset=prt.offset + 512 * h,
                                        ap=[list(prt.ap[0]), [1024, 2],
                                            [1, 512]]),
                                start=(kp == 0), stop=(kp == 7),
                                perf_mode=DR)

                # drain: cast attnout to bf16, ship per-dest + f32 sums
                a2a_t = a2a_in[b][qs][:]
                a2a_f = a2a_t.bitcast(F32)
                for h, acc in ((0, accA), (1, accB)):
                    st = spool.tile([64, 512], BF16, tag="st",
                                    name=f"st_{b}_{qs}_{h}")
                    nc.vector.tensor_copy(st[:], acc[0:64, :])
                    stp = st[:]
                    # payload: st[0:64, 64*d+t] -> a2a[d, 64h+r, t]
                    nc.gpsimd.dma_start(
                        out=bass.AP(tensor=a2a_t.tensor, offset=a2a_t.offset
                                    + 64 * h * 64,
                                    ap=[[64, 64], [CH_BF, 8], [1, 64]]),
                        in_=bass.AP(tensor=stp.tensor, offset=stp.offset,
                                    ap=[list(stp.ap[0]), [64, 8], [1, 64]]))
                    # f32 sums: acc[64, 64*d+t] -> f32 rows 128+2h..129+2h
                    sm_sb = spool.tile([1, 512], F32, tag="sm",
                                       name=f"sm_{b}_{qs}_{h}")
                    nc.vector.tensor_copy(sm_sb[:], acc[64:65, :])
                    sm = sm_sb[:]
                    nc.gpsimd.dma_start(
                        out=bass.AP(tensor=a2a_f.tensor, offset=a2a_f.offset
                                    + (128 + 2 * h) * 32,
                                    ap=[[CH_F32, 8], [1, 64]]),
                        in_=bass.AP(tensor=sm.tensor, offset=sm.offset,
                                    ap=[list(sm.ap[0]), [64, 8], [1, 64]]))
                nc.gpsimd.collective_compute(
                    "AllToAll", mybir.AluOpType.bypass,
                    replica_groups=[list(range(8))],
                    ins=[a2a_in[b][qs][:]], outs=[a2a_out[b][qs][:]])

            # ---- output side ----
            an_all = {}

            def emit_recv(b, p, half):
                """After A2A (b, qs=2p+half): read+normalize into an_all."""
                qs = 2 * p + half
                a2a_t = a2a_out[b][qs][:]
                a2a_f = a2a_t.bitcast(F32)
                if (b, p) not in an_all:
                    an_all[(b, p)] = opool.tile([128, 1024], BF16, tag="an",
                                                name=f"an{b}_{p}")
                raw = opool.tile([128, 512], BF16, tag="raw",
                                 name=f"raw{b}_{qs}")
                rawap = raw[:]
                nc.sync.dma_start(
                    out=bass.AP(tensor=rawap.tensor, offset=rawap.offset,
                                ap=[list(rawap.ap[0]), [64, 8], [1, 64]]),
                    in_=bass.AP(tensor=a2a_t.tensor, offset=a2a_t.offset,
                                ap=[[64, 128], [CH_BF, 8], [1, 64]]))
                sbc = opool.tile([128, 512], F32, tag="sbc",
                                 name=f"sbc{b}_{qs}")
                for h in range(2):
                    sbch = sbc[64 * h:64 * (h + 1), :]
                    nc.sync.dma_start(
                        out=bass.AP(tensor=sbch.tensor, offset=sbch.offset,
                                    ap=[list(sbch.ap[0]), [64, 8], [1, 64]]),
                        in_=bass.AP(tensor=a2a_f.tensor, offset=a2a_f.offset
                                    + (128 + 2 * h) * 32,
                                    ap=[[0, 64], [CH_F32, 8], [1, 64]]))
                rbc = opool.tile([128, 512], F32, tag="rbc",
                                 name=f"rbc{b}_{qs}")
                nc.vector.reciprocal_approx_fast(rbc[:], sbc[:])
                # an[:, 128j + 64*half + t] = raw[:, 64j+t] * rbc[:, 64j+t]
                anap = an_all[(b, p)][:]
                rbcap = rbc[:]
                nc.vector.tensor_tensor(
                    out=bass.AP(tensor=anap.tensor,
                                offset=anap.offset + 64 * half,
                                ap=[list(anap.ap[0]), [128, 8], [1, 64]]),
                    in0=bass.AP(tensor=rawap.tensor, offset=rawap.offset,
                                ap=[list(rawap.ap[0]), [64, 8], [1, 64]]),
                    in1=bass.AP(tensor=rbcap.tensor, offset=rbcap.offset,
                                ap=[list(rbcap.ap[0]), [64, 8], [1, 64]]),
                    op=mybir.AluOpType.mult)

            wo_sb = []
            res_sb = []

            def emit_out_u(b, p, co):
                """Outproj half: 512 out cols for 128 tokens of pair p."""
                an = an_all[(b, p)]
                ps = psum.tile([128, 512], F32, tag="aux", bufs=2,
                               name=f"op{b}_{p}_{co}")
                for j in range(8):
                    nc.tensor.matmul(
                        ps[:], an[:, 128 * j:128 * (j + 1)],
                        wo_sb[j][:, 512 * co:512 * (co + 1)],
                        start=(j == 0), stop=(j == 7))
                ob = opool.tile([128, 512], F32, tag="ob",
                                name=f"ob{b}_{p}_{co}")
                nc.vector.tensor_tensor(
                    out=ob[:], in0=ps[:],
                    in1=res_sb[2 * b + p][:, 512 * co:512 * (co + 1)],
                    op=mybir.AluOpType.add)
                out_t = out1 if b == 0 else out2
                nc.sync.dma_start(
                    out=out_t[128 * p:128 * (p + 1),
                              512 * co:512 * (co + 1)],
                    in_=ob[:])

            # ---------------- emission ----------------
            emit_proj_qk(0, 0, 0)
            emit_proj_qk(0, 0, 1)
            emit_proj_qk(0, 1, 0)
            emit_proj_v(0, 0)
            emit_proj_v(0, 1)

            def qk_u(b, t, j2):
                return lambda: emit_proj_qk(b, t, j2)

            def v_u(b, i):
                return lambda: emit_proj_v(b, i)

            def nop():
                pass

            fill = [qk_u(0, 1, 1), v_u(0, 2),
                    qk_u(0, 1, 2), v_u(0, 3),
                    qk_u(0, 1, 3), v_u(0, 4),
                    qk_u(0, 1, 4), v_u(0, 5),
                    qk_u(0, 1, 5), v_u(0, 6),
                    qk_u(0, 1, 6), v_u(0, 7),
                    qk_u(0, 1, 7), v_u(0, 8),
                    v_u(0, 9), v_u(0, 10),
                    v_u(0, 11), v_u(0, 12),
                    v_u(0, 13), v_u(0, 14),
                    v_u(0, 15), qk_u(0, 0, 2),
                    qk_u(0, 0, 3), qk_u(0, 0, 4),
                    qk_u(0, 0, 5), qk_u(0, 0, 6),
                    qk_u(0, 0, 7)]
            emit_attention_qs(0, 0, fill)
            assert not fill

            # wo / res load (sync queue)
            for cc in range(8):
                t = hpool.tile([128, 1024], BF16, tag=f"wo{cc}",
                               name=f"wo{cc}")
                nc.sync.dma_start(out=t[:],
                                  in_=wo[128 * cc:128 * (cc + 1), :])
                wo_sb.append(t)
            for st_i in range(4):
                t = wpool.tile([128, 1024], BF16, tag=f"res{st_i}",
                               name=f"res{st_i}")
                nc.sync.dma_start(out=t[:],
                                  in_=res[128 * st_i:128 * (st_i + 1), :])
                res_sb.append(t)

            # b1 projections fill b0 qs1/qs2 (hs1 lands ~35us in)
            fill = []
            for j2 in range(8):
                fill.append(qk_u(1, 1, j2))
                fill.append(v_u(1, 2 * (j2 % 4) + (0 if j2 < 4 else 1)))
            emit_attention_qs(0, 1, fill)
            fill = []
            for j2 in range(8):
                fill.append(qk_u(1, 0, j2))
                fill.append(v_u(1, 8 + 2 * (j2 % 4) + (0 if j2 < 4 else 1)))
            emit_attention_qs(0, 2, fill)
            emit_attention_qs(0, 3, [])

            emit_attention_qs(1, 0, [])
            emit_attention_qs(1, 1, [])
            emit_recv(0, 0, 0)
            emit_recv(0, 0, 1)
            fill = [nop] * 10 + [lambda: emit_out_u(0, 0, 0),
                                 lambda: emit_out_u(0, 0, 1)]
            emit_attention_qs(1, 2, fill)
            emit_recv(0, 1, 0)
            emit_recv(0, 1, 1)
            fill = [nop] * 10 + [lambda: emit_out_u(0, 1, 0),
                                 lambda: emit_out_u(0, 1, 1)]
            emit_attention_qs(1, 3, fill)
            # tail: out(1,0) doubles as PE warm-keeper during last A2A
            emit_recv(1, 0, 0)
            emit_recv(1, 0, 1)
            emit_out_u(1, 0, 0)
            emit_out_u(1, 0, 1)
            warm = psum.tile([128, 512], F32, tag="aux", bufs=2,
                             name="warm")
            for wi in range(10):
                nc.tensor.matmul(warm[:], wo_sb[0][:, 0:128],
                                 wo_sb[1][:, 0:512],
                                 start=True, stop=True,
                                 skip_group_check=True)
            emit_recv(1, 1, 0)
            emit_recv(1, 1, 1)
            emit_out_u(1, 1, 0)
            emit_out_u(1, 1, 1)
    nc.finalize()
    return nc


def _prep_inputs(hidden_states, Wq, bq, Wk, bk, Wv, bv, Wo, bo):
    import ml_dtypes
    bf16 = ml_dtypes.bfloat16
    fp8 = ml_dtypes.float8_e4m3fn
    hs = np.asarray(hidden_states, np.float32)
    hsT = np.clip(np.ascontiguousarray(
        hs.transpose(2, 0, 1).reshape(C, BS)), -240, 240).astype(fp8)
    Wo_h = np.ascontiguousarray(np.asarray(Wo, np.float32)).astype(bf16)
    bo_f = np.asarray(bo, np.float32)

    def pack_pairs(w, ncols, stride):
        """[C, ncols] -> [128, 8*stride]: col 2*stride*mp + stride*i + m
        = WS * w[128*(2*mp+i) + p, m], fp8."""
        out = np.zeros((128, 8 * stride), np.float32)
        for mp in range(4):
            for i in range(2):
                blk = w[128 * (2 * mp + i):128 * (2 * mp + i + 1), :]
                out[:, 2 * stride * mp + stride * i:
                    2 * stride * mp + stride * i + ncols] = WS * blk
        return np.clip(out, -240, 240).astype(fp8)

    in_maps = []
    for c in range(N_CORES):
        h0 = 2 * c
        cols = slice(64 * h0, 64 * h0 + 128)
        wv_c = np.zeros((C, 130), np.float32)
        bvb_c = np.zeros((1, 130), np.float32)
        for a in range(2):
            hd = slice(64 * (h0 + a), 64 * (h0 + a + 1))
            wv_c[:, 65 * a:65 * a + 64] = np.asarray(Wv, np.float32)[:, hd]
            bvb_c[0, 65 * a:65 * a + 64] = WS * np.asarray(
                bv, np.float32)[hd]
            bvb_c[0, 65 * a + 64] = WS  # ones column x32: sums match v x32
        bqk_c = WS * np.stack([np.asarray(bq, np.float32)[cols],
                               np.asarray(bk, np.float32)[cols]], axis=1)
        res_c = np.empty((512, C), np.float32)
        for b in range(2):
            for qs in range(4):
                rows = slice(64 * (4 * b + qs), 64 * (4 * b + qs) + 64)
                toks = slice(512 * qs + 64 * c, 512 * qs + 64 * c + 64)
                res_c[rows] = hs[b, toks, :] + bo_f
        in_maps.append({
            "hsT": hsT,
            "wq": pack_pairs(np.asarray(Wq, np.float32)[:, cols], 128, 128),
            "wk": pack_pairs(np.asarray(Wk, np.float32)[:, cols], 128, 128),
            "wv": pack_pairs(wv_c, 130, 144),
            "wo": Wo_h,
            "bqk": np.ascontiguousarray(bqk_c),
            "bvb": bvb_c,
            "res": np.ascontiguousarray(res_c).astype(bf16),
        })
    return in_maps


def _run(inputs, trace=False, trace_kwargs=None):
    if "nc" not in _CACHE:
        _CACHE["nc"] = _build()
    nc = _CACHE["nc"]
    in_maps = _prep_inputs(**inputs)
    r = run_bass_kernel_spmd(nc, in_maps, core_ids=list(range(N_CORES)),
                             trace=trace, **(trace_kwargs or {}))
    full = np.empty((B, S, C), np.float32)
    for c in range(N_CORES):
        for b in range(2):
            o = r.results[c]["out1" if b == 0 else "out2"]
            for qs in range(4):
                full[b, 512 * qs + 64 * c:512 * qs + 64 * c + 64, :] = \
                    o[64 * qs:64 * qs + 64]
    return full, r


def kernel(**inputs):
    full, _ = _run(inputs, trace=False)
    return full


# revision 17
# speedup vs baseline: 1.1633x; 1.1067x over previous
"""Trainium2 Bass kernel for AttnProcessor self-attention (B=2,S=2048,C=1024,H=16).

Sharding: 8 cores, core c owns heads (2c, 2c+1) for both batches (tensor
parallel on the head dim for QKV). Projections run in fp8e4 DoubleRow
(hs and x32-scaled weights; the ones-column carries 32.0 so softmax
normalization absorbs the scale); QK runs bf16; PV runs fp8 DoubleRow
(probs e5m2, v e4m3); output projection bf16. rel err ~8e-3 vs 2e-2 gate.

Token->core output mapping is interleaved so each 512-token q-slice (qs)
contains one 64-token block for every destination core: core c owns tokens
512*qs + 64*c .. +64 of every (b, qs). Each (b, qs) ships its attention
output in its own small AllToAll ([8,132,64] bf16; rows 128-131 carry the
f32 softmax sums, bitcast) right after that q-slice's PV completes - 8
pipelined collectives. Reciprocals run receiver-side with
reciprocal_approx_fast; output projection runs per qs-pair as PE fill
work inside later attention slices / the tail.

b0's hs arrives in 512-token column slices so the projection pipeline
starts after ~1/4 of the load. ScalarE runs ONLY exps (plus a few early
load DMAs); DMAs are issued from sync/scalar/gpsimd rings; DVE does
casts/recips/biases/normalization.
"""
import numpy as np

import concourse.bacc as bacc
import concourse.bass as bass
import concourse.tile as tile
import concourse.tile_rust as tile_rust
from concourse import mybir
from concourse.bass_utils import run_bass_kernel_spmd

F32 = mybir.dt.float32
BF16 = mybir.dt.bfloat16
FP8E4 = mybir.dt.float8e4
FP8E5 = mybir.dt.float8e5

B, S, C, H, D = 2, 2048, 1024, 16, 64
N_CORES = 8
BS = B * S  # 4096
SCALE = 1.0 / np.sqrt(D)
WS = 32.0  # fp8 projection weight scale (weights are sigma=1/32)

# a2a chunk geometry (per destination core): 132 rows x 64 tokens bf16
CH_BF = 132 * 64          # bf16 elems per dest chunk (8448)
CH_F32 = CH_BF // 2       # f32 elems per dest chunk (4224)

_CACHE = {}
DR = mybir.MatmulPerfMode.DoubleRow


def _build():
    nc = bacc.Bacc(num_devices=N_CORES)
    hsT = nc.declare_dram_parameter("hsT", [C, BS], FP8E4, isOutput=False)
    wq = nc.declare_dram_parameter("wq", [128, 1024], FP8E4, isOutput=False)
    wk = nc.declare_dram_parameter("wk", [128, 1024], FP8E4, isOutput=False)
    wv = nc.declare_dram_parameter("wv", [128, 1152], FP8E4, isOutput=False)
    wo = nc.declare_dram_parameter("wo", [C, C], BF16, isOutput=False)
    bqk = nc.declare_dram_parameter("bqk", [128, 2], F32, isOutput=False)
    bvb = nc.declare_dram_parameter("bvb", [1, 130], F32, isOutput=False)
    res = nc.declare_dram_parameter("res", [512, C], BF16, isOutput=False)
    out1 = nc.declare_dram_parameter("out1", [256, C], F32, isOutput=True)
    out2 = nc.declare_dram_parameter("out2", [256, C], F32, isOutput=True)

    with tile.TileContext(nc) as tc:
        with (
            tc.tile_pool(name="wpool", bufs=1) as wpool,
            tc.tile_pool(name="hpool", bufs=1) as hpool,
            tc.tile_pool(name="qkpool", bufs=2) as qkpool,
            tc.tile_pool(name="ppool", bufs=4) as ppool,
            tc.tile_pool(name="spool", bufs=4) as spool,
            tc.tile_pool(name="opool", bufs=2) as opool,
            tc.tile_pool(name="psum", bufs=1, space="PSUM") as psum,
            tc.tile_pool(name="dram", bufs=1, space="DRAM") as dram,
        ):
            # ---- weight / constant loads ----
            wq_sb = wpool.tile([128, 1024], FP8E4, tag="wq")
            nc.sync.dma_start(out=wq_sb[:], in_=wq[:])
            wk_sb = wpool.tile([128, 1024], FP8E4, tag="wk")
            nc.scalar.dma_start(out=wk_sb[:], in_=wk[:])
            wv_sb = wpool.tile([128, 1152], FP8E4, tag="wv")
            nc.gpsimd.dma_start(out=wv_sb[:], in_=wv[:])
            bqk_sb = wpool.tile([128, 2], F32, tag="bqk")
            nc.gpsimd.dma_start(out=bqk_sb[:], in_=bqk[:])
            bvb_sb = wpool.tile([128, 130], F32, tag="bvb")
            bvb_ap = bvb[:]
            nc.gpsimd.dma_start(
                out=bvb_sb[:],
                in_=bass.AP(tensor=bvb_ap.tensor, offset=bvb_ap.offset,
                            ap=[[0, 128], [1, 130]]))

            # b0 hs: chunk-pair tiles [128, 2x512] per (mp, j4), streamed
            # token-slice-major across 3 rings
            rings = [nc.sync, nc.scalar, nc.gpsimd]
            hs0 = [[None] * 4 for _ in range(4)]
            hs0_dmas = []
            n = 0
            for j4 in range(4):
                for mp in range(4):
                    t = hpool.tile([128, 1024], FP8E4, tag=f"hs0_{mp}_{j4}",
                                   name=f"hs0_{mp}_{j4}")
                    for i in range(2):
                        d = rings[n % 3].dma_start(
                            out=t[:, 512 * i:512 * (i + 1)],
                            in_=hsT[128 * (2 * mp + i):
                                    128 * (2 * mp + i + 1),
                                    512 * j4:512 * (j4 + 1)])
                        hs0_dmas.append(d)
                        n += 1
                    hs0[mp][j4] = t
            hs1 = []
            for mp in range(4):
                t = hpool.tile([128, 4096], FP8E4, tag=f"hs1_{mp}",
                               name=f"hs1_{mp}")
                for i in range(2):
                    d = [nc.sync, nc.scalar][(mp + i) % 2].dma_start(
                        out=t[:, 2048 * i:2048 * (i + 1)],
                        in_=hsT[128 * (2 * mp + i):128 * (2 * mp + i + 1),
                                2048:4096])
                    for a in hs0_dmas[-3:]:
                        tile_rust.add_dep_helper(
                            d.ins, a.ins, True, "hs1 after hs0 (bandwidth)")
                hs1.append(t)

            a2a_in = [[dram.tile([8, 132, 64], BF16, name=f"a2ain{b}_{qs}")
                       for qs in range(4)] for b in range(2)]
            a2a_out = [[dram.tile([8, 132, 64], BF16, name=f"a2aout{b}_{qs}")
                        for qs in range(4)] for b in range(2)]

            qT, kT, vS = {}, {}, {}

            def hs_rhs(b, mp, col, width):
                """fp8 DR AP for hs chunk-pair mp, token cols [col,col+w)."""
                if b == 0:
                    t = hs0[mp][col // 512][:]
                    off = col % 512
                    return bass.AP(tensor=t.tensor, offset=t.offset + off,
                                   ap=[list(t.ap[0]), [512, 2], [1, width]])
                t = hs1[mp][:]
                return bass.AP(tensor=t.tensor, offset=t.offset + col,
                               ap=[list(t.ap[0]), [2048, 2], [1, width]])

            def emit_proj_qk(b, t_idx, j2):
                """One unit: tensor t_idx (0=q,1=k), one 256-wide s-slice.
                fp8 DoubleRow over chunk pairs."""
                if t_idx == 0:
                    if b not in qT:
                        qT[b] = qkpool.tile([128, 2048], BF16, tag="qT",
                                            name=f"qT{b}")
                    dst, w_sb = qT[b], wq_sb
                else:
                    if b not in kT:
                        kT[b] = qkpool.tile([128, 2048], BF16, tag="kT",
                                            name=f"kT{b}")
                    dst, w_sb = kT[b], wk_sb
                ps = psum.tile([128, 512], F32, tag="aux", bufs=2,
                               name=f"pqk{b}_{t_idx}_{j2}")
                sl = ps[:, 0:256]
                wap = w_sb[:]
                for mp in range(4):
                    nc.tensor.matmul(
                        sl,
                        bass.AP(tensor=wap.tensor,
                                offset=wap.offset + 256 * mp,
                                ap=[list(wap.ap[0]), [128, 2], [1, 128]]),
                        hs_rhs(b, mp, 256 * j2, 256),
                        start=(mp == 0), stop=(mp == 3), perf_mode=DR)
                nc.vector.tensor_scalar_add(
                    out=dst[:, 256 * j2:256 * (j2 + 1)], in0=sl,
                    scalar1=bqk_sb[:, t_idx:t_idx + 1])

            def emit_proj_v(b, i):
                """One unit: one 128-row v' s-tile i -> fp8e4 vS (x32).
                vS layout (PV DoubleRow pairs): pair kc'=i//2 block at
                320*kc', head h at +160*h, parity i%2 at +80."""
                if b not in vS:
                    vS[b] = qkpool.tile([128, 2560], FP8E4, tag="vS",
                                        name=f"vS{b}")
                ps = psum.tile([128, 512], F32, tag="aux", bufs=2,
                               name=f"pv{b}_{i}")
                sl = ps[:, 0:144]
                wap = wv_sb[:]
                for mp in range(4):
                    nc.tensor.matmul(
                        sl,
                        hs_rhs(b, mp, 128 * i, 128),
                        bass.AP(tensor=wap.tensor,
                                offset=wap.offset + 288 * mp,
                                ap=[list(wap.ap[0]), [144, 2], [1, 144]]),
                        start=(mp == 0), stop=(mp == 3), perf_mode=DR)
                vt = vS[b][:]
                bvt = bvb_sb[:]
                slb = ps[:, 0:130]
                nc.vector.tensor_tensor(
                    out=bass.AP(tensor=vt.tensor, offset=vt.offset
                                + 320 * (i // 2) + 80 * (i % 2),
                                ap=[list(vt.ap[0]), [160, 2], [1, 65]]),
                    in0=bass.AP(tensor=slb.tensor, offset=slb.offset,
                                ap=[list(slb.ap[0]), [65, 2], [1, 65]]),
                    in1=bass.AP(tensor=bvt.tensor, offset=bvt.offset,
                                ap=[list(bvt.ap[0]), [65, 2], [1, 65]]),
                    op=mybir.AluOpType.add)

            def emit_attention_qs(b, qs, fill_work):
                """One q-slice (512 q) for both heads; 16 kc steps.
                Per step: 2 fills, exp(kc), QK(kc+1); PV (DoubleRow,
                paired kc blocks) after odd kc."""
                accA = psum.tile([65, 512], F32, tag="accA", bufs=1,
                                 name=f"accA_{b}_{qs}")
                accB = psum.tile([65, 512], F32, tag="accB", bufs=1,
                                 name=f"accB_{b}_{qs}")
                sc_t = {}

                def emit_qk(kc):
                    sc = psum.tile([128, 1024], F32, tag="sc", bufs=2,
                                   name=f"sc_{b}_{qs}_{kc}")
                    sc_t[kc] = sc
                    nc.tensor.matmul(
                        sc[:, 0:512],
                        kT[b][0:64, 128 * kc:128 * (kc + 1)],
                        qT[b][0:64, 512 * qs:512 * (qs + 1)],
                        start=True, stop=True)
                    nc.tensor.matmul(
                        sc[:, 512:1024],
                        kT[b][64:128, 128 * kc:128 * (kc + 1)],
                        qT[b][64:128, 512 * qs:512 * (qs + 1)],
                        start=True, stop=True)

                emit_qk(0)
                pr2 = None
                for kc in range(16):
                    for _ in range(2):
                        if fill_work:
                            fill_work.pop(0)()
                    if kc % 2 == 0:
                        pr2 = ppool.tile([128, 2048], FP8E5, tag="pr",
                                         bufs=2, name=f"pr_{b}_{qs}_{kc}")
                    nc.scalar.activation(
                        pr2[:, 1024 * (kc % 2):1024 * (kc % 2 + 1)],
                        sc_t.pop(kc)[:],
                        mybir.ActivationFunctionType.Exp,
                        scale=float(SCALE / (WS * WS)))
                    if kc < 15:
                        emit_qk(kc + 1)
                    if kc % 2 == 1:
                        kp = kc // 2
                        vt = vS[b][:]
                        prt = pr2[:]
                        for h, acc in ((0, accA), (1, accB)):
                            nc.tensor.matmul(
                                acc[:],
                                bass.AP(tensor=vt.tensor, offset=vt.offset
                                        + 320 * kp + 160 * h,
                                        ap=[list(vt.ap[0]), [80, 2],
                                            [1, 65]]),
                                bass.AP(tensor=prt.tensor,
                                        offset=prt.offset + 512 * h,
                                        ap=[list(prt.ap[0]), [1024, 2],
                                            [1, 512]]),
                                start=(kp == 0), stop=(kp == 7),
                                perf_mode=DR)

                # drain: cast attnout to bf16, ship per-dest + f32 sums
                a2a_t = a2a_in[b][qs][:]
                a2a_f = a2a_t.bitcast(F32)
                for h, acc in ((0, accA), (1, accB)):
                    st = spool.tile([64, 512], BF16, tag="st",
                                    name=f"st_{b}_{qs}_{h}")
                    nc.vector.tensor_copy(st[:], acc[0:64, :])
                    stp = st[:]
                    # payload: st[0:64, 64*d+t] -> a2a[d, 64h+r, t]
                    nc.gpsimd.dma_start(
                        out=bass.AP(tensor=a2a_t.tensor, offset=a2a_t.offset
                                    + 64 * h * 64,
                                    ap=[[64, 64], [CH_BF, 8], [1, 64]]),
                        in_=bass.AP(tensor=stp.tensor, offset=stp.offset,
                                    ap=[list(stp.ap[0]), [64, 8], [1, 64]]))
                    # f32 sums: acc[64, 64*d+t] -> f32 rows 128+2h..129+2h
                    sm_sb = spool.tile([1, 512], F32, tag="sm",
                                       name=f"sm_{b}_{qs}_{h}")
                    nc.vector.tensor_copy(sm_sb[:], acc[64:65, :])
                    sm = sm_sb[:]
                    nc.gpsimd.dma_start(
                        out=bass.AP(tensor=a2a_f.tensor, offset=a2a_f.offset
                                    + (128 + 2 * h) * 32,
                                    ap=[[CH_F32, 8], [1, 64]]),
                        in_=bass.AP(tensor=sm.tensor, offset=sm.offset,
                                    ap=[list(sm.ap[0]), [64, 8], [1, 64]]))
                nc.gpsimd.collective_compute(
                    "AllToAll", mybir.AluOpType.bypass,
                    replica_groups=[list(range(8))],
                    ins=[a2a_in[b][qs][:]], outs=[a2a_out[b][qs][:]])

            # ---- output side ----
            an_all = {}

            def emit_recv(b, p, half):
                """After A2A (b, qs=2p+half): read+normalize into an_all."""
                qs = 2 * p + half
                a2a_t = a2a_out[b][qs][:]
                a2a_f = a2a_t.bitcast(F32)
                if (b, p) not in an_all:
                    an_all[(b, p)] = opool.tile([128, 1024], BF16, tag="an",
                                                name=f"an{b}_{p}")
                raw = opool.tile([128, 512], BF16, tag="raw",
                                 name=f"raw{b}_{qs}")
                rawap = raw[:]
                nc.sync.dma_start(
                    out=bass.AP(tensor=rawap.tensor, offset=rawap.offset,
                                ap=[list(rawap.ap[0]), [64, 8], [1, 64]]),
                    in_=bass.AP(tensor=a2a_t.tensor, offset=a2a_t.offset,
                                ap=[[64, 128], [CH_BF, 8], [1, 64]]))
                sbc = opool.tile([128, 512], F32, tag="sbc",
                                 name=f"sbc{b}_{qs}")
                for h in range(2):
                    sbch = sbc[64 * h:64 * (h + 1), :]
                    nc.sync.dma_start(
                        out=bass.AP(tensor=sbch.tensor, offset=sbch.offset,
                                    ap=[list(sbch.ap[0]), [64, 8], [1, 64]]),
                        in_=bass.AP(tensor=a2a_f.tensor, offset=a2a_f.offset
                                    + (128 + 2 * h) * 32,
                                    ap=[[0, 64], [CH_F32, 8], [1, 64]]))
                rbc = opool.tile([128, 512], F32, tag="rbc",
                                 name=f"rbc{b}_{qs}")
                nc.vector.reciprocal_approx_fast(rbc[:], sbc[:])
                # an[:, 128j + 64*half + t] = raw[:, 64j+t] * rbc[:, 64j+t]
                anap = an_all[(b, p)][:]
                rbcap = rbc[:]
                nc.vector.tensor_tensor(
                    out=bass.AP(tensor=anap.tensor,
                                offset=anap.offset + 64 * half,
                                ap=[list(anap.ap[0]), [128, 8], [1, 64]]),
                    in0=bass.AP(tensor=rawap.tensor, offset=rawap.offset,
                                ap=[list(rawap.ap[0]), [64, 8], [1, 64]]),
                    in1=bass.AP(tensor=rbcap.tensor, offset=rbcap.offset,
                                ap=[list(rbcap.ap[0]), [64, 8], [1, 64]]),
                    op=mybir.AluOpType.mult)

            wo_sb = []
            res_sb = []

            out_ps = {}

            def emit_out_mm(b, p, co):
                """Outproj half matmuls: 512 cols for 128 tokens of pair."""
                an = an_all[(b, p)]
                ps = psum.tile([128, 512], F32, tag="aux", bufs=2,
                               name=f"op{b}_{p}_{co}")
                for j in range(8):
                    nc.tensor.matmul(
                        ps[:], an[:, 128 * j:128 * (j + 1)],
                        wo_sb[j][:, 512 * co:512 * (co + 1)],
                        start=(j == 0), stop=(j == 7))
                out_ps[(b, p, co)] = ps

            def emit_out_fin(b, p, co):
                """Residual add + store (DVE+sync). Emitted late so the
                psum read never sits ahead of drain casts in the DVE FIFO."""
                ps = out_ps.pop((b, p, co))
                ob = opool.tile([128, 512], F32, tag="ob",
                                name=f"ob{b}_{p}_{co}")
                nc.vector.tensor_tensor(
                    out=ob[:], in0=ps[:],
                    in1=res_sb[2 * b + p][:, 512 * co:512 * (co + 1)],
                    op=mybir.AluOpType.add)
                out_t = out1 if b == 0 else out2
                nc.sync.dma_start(
                    out=out_t[128 * p:128 * (p + 1),
                              512 * co:512 * (co + 1)],
                    in_=ob[:])

            # ---------------- emission ----------------
            emit_proj_qk(0, 0, 0)
            emit_proj_qk(0, 0, 1)
            emit_proj_qk(0, 1, 0)
            emit_proj_v(0, 0)
            emit_proj_v(0, 1)

            def qk_u(b, t, j2):
                return lambda: emit_proj_qk(b, t, j2)

            def v_u(b, i):
                return lambda: emit_proj_v(b, i)

            def nop():
                pass

            fill = [qk_u(0, 1, 1), v_u(0, 2),
                    qk_u(0, 1, 2), v_u(0, 3),
                    qk_u(0, 1, 3), v_u(0, 4),
                    qk_u(0, 1, 4), v_u(0, 5),
                    qk_u(0, 1, 5), v_u(0, 6),
                    qk_u(0, 1, 6), v_u(0, 7),
                    qk_u(0, 1, 7), v_u(0, 8),
                    v_u(0, 9), v_u(0, 10),
                    v_u(0, 11), v_u(0, 12),
                    v_u(0, 13), v_u(0, 14),
                    v_u(0, 15), qk_u(0, 0, 2),
                    qk_u(0, 0, 3), qk_u(0, 0, 4),
                    qk_u(0, 0, 5), qk_u(0, 0, 6),
                    qk_u(0, 0, 7)]
            emit_attention_qs(0, 0, fill)
            assert not fill

            # wo / res load (sync queue)
            for cc in range(8):
                t = hpool.tile([128, 1024], BF16, tag=f"wo{cc}",
                               name=f"wo{cc}")
                nc.sync.dma_start(out=t[:],
                                  in_=wo[128 * cc:128 * (cc + 1), :])
                wo_sb.append(t)
            for st_i in range(4):
                t = wpool.tile([128, 1024], BF16, tag=f"res{st_i}",
                               name=f"res{st_i}")
                nc.sync.dma_start(out=t[:],
                                  in_=res[128 * st_i:128 * (st_i + 1), :])
                res_sb.append(t)

            # b1 projections fill b0 qs1/qs2 (hs1 lands ~35us in)
            fill = []
            for j2 in range(8):
                fill.append(qk_u(1, 1, j2))
                fill.append(v_u(1, 2 * (j2 % 4) + (0 if j2 < 4 else 1)))
            emit_attention_qs(0, 1, fill)
            fill = []
            for j2 in range(8):
                fill.append(qk_u(1, 0, j2))
                fill.append(v_u(1, 8 + 2 * (j2 % 4) + (0 if j2 < 4 else 1)))
            emit_attention_qs(0, 2, fill)
            emit_attention_qs(0, 3, [])

            emit_attention_qs(1, 0, [])
            emit_attention_qs(1, 1, [])
            emit_recv(0, 0, 0)
            emit_recv(0, 0, 1)
            fill = [nop] * 10 + [lambda: emit_out_mm(0, 0, 0),
                                 lambda: emit_out_mm(0, 0, 1)]
            emit_attention_qs(1, 2, fill)
            emit_out_fin(0, 0, 0)
            emit_out_fin(0, 0, 1)
            emit_recv(0, 1, 0)
            emit_recv(0, 1, 1)
            fill = [nop] * 10 + [lambda: emit_out_mm(0, 1, 0),
                                 lambda: emit_out_mm(0, 1, 1)]
            emit_attention_qs(1, 3, fill)
            emit_out_fin(0, 1, 0)
            emit_out_fin(0, 1, 1)
            # tail: out(1,0) doubles as PE warm-keeper during last A2A
            emit_recv(1, 0, 0)
            emit_recv(1, 0, 1)
            emit_out_mm(1, 0, 0)
            emit_out_fin(1, 0, 0)
            emit_out_mm(1, 0, 1)
            emit_out_fin(1, 0, 1)
            warm = psum.tile([128, 512], F32, tag="aux", bufs=2,
                             name="warm")
            for wi in range(10):
                nc.tensor.matmul(warm[:], wo_sb[0][:, 0:128],
                                 wo_sb[1][:, 0:512],
                                 start=True, stop=True,
                                 skip_group_check=True)
            emit_recv(1, 1, 0)
            emit_recv(1, 1, 1)
            emit_out_mm(1, 1, 0)
            emit_out_fin(1, 1, 0)
            emit_out_mm(1, 1, 1)
            emit_out_fin(1, 1, 1)
    nc.finalize()
    return nc


def _prep_inputs(hidden_states, Wq, bq, Wk, bk, Wv, bv, Wo, bo):
    import ml_dtypes
    bf16 = ml_dtypes.bfloat16
    fp8 = ml_dtypes.float8_e4m3fn
    hs = np.asarray(hidden_states, np.float32)
    hsT = np.clip(np.ascontiguousarray(
        hs.transpose(2, 0, 1).reshape(C, BS)), -240, 240).astype(fp8)
    Wo_h = np.ascontiguousarray(np.asarray(Wo, np.float32)).astype(bf16)
    bo_f = np.asarray(bo, np.float32)

    def pack_pairs(w, ncols, stride):
        """[C, ncols] -> [128, 8*stride]: col 2*stride*mp + stride*i + m
        = WS * w[128*(2*mp+i) + p, m], fp8."""
        out = np.zeros((128, 8 * stride), np.float32)
        for mp in range(4):
            for i in range(2):
                blk = w[128 * (2 * mp + i):128 * (2 * mp + i + 1), :]
                out[:, 2 * stride * mp + stride * i:
                    2 * stride * mp + stride * i + ncols] = WS * blk
        return np.clip(out, -240, 240).astype(fp8)

    in_maps = []
    for c in range(N_CORES):
        h0 = 2 * c
        cols = slice(64 * h0, 64 * h0 + 128)
        wv_c = np.zeros((C, 130), np.float32)
        bvb_c = np.zeros((1, 130), np.float32)
        for a in range(2):
            hd = slice(64 * (h0 + a), 64 * (h0 + a + 1))
            wv_c[:, 65 * a:65 * a + 64] = np.asarray(Wv, np.float32)[:, hd]
            bvb_c[0, 65 * a:65 * a + 64] = WS * np.asarray(
                bv, np.float32)[hd]
            bvb_c[0, 65 * a + 64] = WS  # ones column x32: sums match v x32
        bqk_c = WS * np.stack([np.asarray(bq, np.float32)[cols],
                               np.asarray(bk, np.float32)[cols]], axis=1)
        res_c = np.empty((512, C), np.float32)
        for b in range(2):
            for qs in range(4):
                rows = slice(64 * (4 * b + qs), 64 * (4 * b + qs) + 64)
                toks = slice(512 * qs + 64 * c, 512 * qs + 64 * c + 64)
                res_c[rows] = hs[b, toks, :] + bo_f
        in_maps.append({
            "hsT": hsT,
            "wq": pack_pairs(np.asarray(Wq, np.float32)[:, cols], 128, 128),
            "wk": pack_pairs(np.asarray(Wk, np.float32)[:, cols], 128, 128),
            "wv": pack_pairs(wv_c, 130, 144),
            "wo": Wo_h,
            "bqk": np.ascontiguousarray(bqk_c),
            "bvb": bvb_c,
            "res": np.ascontiguousarray(res_c).astype(bf16),
        })
    return in_maps


def _run(inputs, trace=False, trace_kwargs=None):
    if "nc" not in _CACHE:
        _CACHE["nc"] = _build()
    nc = _CACHE["nc"]
    in_maps = _prep_inputs(**inputs)
    r = run_bass_kernel_spmd(nc, in_maps, core_ids=list(range(N_CORES)),
                             trace=trace, **(trace_kwargs or {}))
    full = np.empty((B, S, C), np.float32)
    for c in range(N_CORES):
        for b in range(2):
            o = r.results[c]["out1" if b == 0 else "out2"]
            for qs in range(4):
                full[b, 512 * qs + 64 * c:512 * qs + 64 * c + 64, :] = \
                    o[64 * qs:64 * qs + 64]
    return full, r


def kernel(**inputs):
    full, _ = _run(inputs, trace=False)
    return full
